# revision 27
# baseline (speedup 1.0000x reference)
"""Trainium2 Bass kernel for nn_Cross_MultiAttention (8-head cross attention).

Sharding: one attention head per NeuronCore (8 heads / 8 cores).

Host folds the shared 1x1 input conv into each head's q/k/v projections
(Aq = wq_h @ w_in etc.).  Everything is padded to N=5120 tokens; padded key
columns are masked out, padded query columns are sliced away on the host.

Per core:
  - stage "0" interleaves the q/k/v projections with stripe 0's score
    groups so the scalar engine starts exp'ing ~30us earlier.
  - attention scores are computed TRANSPOSED (keys on partitions, queries
    on the free dim) with K=32 contraction row-tiled 2x.
  - softmax is max-free (|scores/16| < ~4); each score group (2 j-tiles)
    takes one of three paths, chosen to balance the PE/ACT/DVE engines:
      AP: PE adds -240*mask into the score PSUM (fp8 negI matmul), then
          the scalar engine exps straight into the fp8 probability store.
      DP: PE adds the mask as in AP; the vector engine then computes the
          fp8 probability BITS directly with a saturating uint8 Schraudolph
          (p8 = u8(s*A8 + B8); masked scores saturate to 0 = fp8 +0.0).
      AD: no PE mask; scalar-engine exp, then the vector engine multiplies
          by a {0,1} keep-mask (the host bakes 1-m into those slab groups).
  - P@V runs in fp8 with DoubleRow perf mode; the denominator comes from an
    appended ones-column in V.
  - per stripe the raw [34, 512] head output (32 dims + denominator) is
    copied PSUM->SBUF and DMA'd out; the small output projection w_out and
    the division happen on the host.
"""

import numpy as np

import concourse.bacc as bacc
import concourse.tile as tile
import concourse.mybir as mybir
from concourse.bass_utils import run_bass_kernel_spmd

F32 = mybir.dt.float32
F16 = mybir.dt.float16
F8 = mybir.dt.float8e4
U8 = mybir.dt.uint8
U16 = mybir.dt.uint16
AF = mybir.ActivationFunctionType
ALU = mybir.AluOpType
DR = mybir.MatmulPerfMode.DoubleRow

EMB = 256
HEADS = 8
DEPTH = 32
IN_CH = 256
H, W = 50, 100
N_TOK = H * W          # 5000
N_PAD = 5120           # padded token count (40 j-tiles of 128, 10 stripes of 512)
WSZ = 512
NW = N_PAD // WSZ      # 10
NJ = N_PAD // 128      # 40
NG = NJ // 2           # 20 score groups per stripe (2 j-tiles each)
NT = N_PAD // 512      # 10 projection tiles
SCALE = EMB ** (-0.5)  # 1/16
NEG = 240.0            # additive mask weight; exp((s-240)/16) flushes to 0 in fp8

# fp8-bit Schraudolph: u8 bits of e4m3(exp(s/16)) ~= sat_u8(s*A8 + B8)
A8 = 8.0 / (16.0 * np.log(2.0))
B8 = 55.53

# per-group softmax path: AP = ACT exp + PE mask, DP = DVE u8-schraudolph +
# PE mask, AD = ACT exp + DVE keep-mask multiply.
AP, DP, AD = 0, 1, 2
# keep the PE the clearly-busiest engine: if it falls below the consumers it
# idles, the HAM clock-gate re-throttles it to 1.2 GHz, and the whole stripe
# pipeline oscillates (measured: 16 HAM flips, +75us).
PAT0 = [AP] * 12 + [AD] * 8                       # stripe 0: AND-masks for late groups, issued deferred (below) to keep the DVE FIFO clear
# stripes 1..9: 2 AP / 8 DP / 10 AD, AD-heavy early so the tail PVs (which
# wait on the last p_store write) aren't gated by a late DVE AND
PATW = [AD, DP, AD, DP, AD, DP, AD, DP, AD, DP,
        AD, DP, AD, AP, AD, DP, AD, DP, AD, AP]
PAT = [PAT0] + [PATW] * (NW - 1)


def build_nc(num_devices=8):
    """Build the Bass module (same SPMD program for every core)."""
    nc = bacc.Bacc("TRN2", target_bir_lowering=False, debug=False,
                   num_devices=num_devices)

    D = DEPTH
    xp_d = nc.dram_tensor("xp", (IN_CH, N_PAD), F16, kind="ExternalInput").ap()
    cp_d = nc.dram_tensor("cp", (IN_CH, N_PAD), F16, kind="ExternalInput").ap()
    nm3_d = nc.dram_tensor("nm3", (128, NW, NJ, WSZ), F8,
                           kind="ExternalInput").ap()
    AqT_d = nc.dram_tensor("AqT", (IN_CH, 4 * D), F16, kind="ExternalInput").ap()
    cq_d = nc.dram_tensor("cq", (4 * D, 1), F32, kind="ExternalInput").ap()
    AkT_d = nc.dram_tensor("AkT", (IN_CH, 4 * D), F16, kind="ExternalInput").ap()
    ck_d = nc.dram_tensor("ck", (4 * D, 1), F32, kind="ExternalInput").ap()
    AvT_d = nc.dram_tensor("AvT", (IN_CH, D), F16, kind="ExternalInput").ap()
    cvb_d = nc.dram_tensor("cvb", (128, D), F32, kind="ExternalInput").ap()
    negI_d = nc.dram_tensor("negI", (128, 128), F8, kind="ExternalInput").ap()
    av_d = nc.dram_tensor("av", (34, N_PAD), F32, kind="ExternalOutput").ap()

    with tile.TileContext(nc) as tc:
        with (
            tc.tile_pool(name="persist", bufs=1) as persist,
            tc.tile_pool(name="consts", bufs=1) as consts,
        ):
            # ---- constants to SBUF ----
            AqT_sb = consts.tile([128, 2, 4 * D], F16)
            AkT_sb = consts.tile([128, 2, 4 * D], F16)
            AvT_sb = consts.tile([128, 2, D], F16)
            for ct in range(2):
                nc.sync.dma_start(AqT_sb[:, ct, :], AqT_d[ct * 128:(ct + 1) * 128, :])
                nc.sync.dma_start(AkT_sb[:, ct, :], AkT_d[ct * 128:(ct + 1) * 128, :])
                nc.sync.dma_start(AvT_sb[:, ct, :], AvT_d[ct * 128:(ct + 1) * 128, :])
            cq_sb = consts.tile([4 * D, 1], F32)
            nc.sync.dma_start(cq_sb[:, :], cq_d[:, :])
            ck_sb = consts.tile([4 * D, 1], F32)
            nc.sync.dma_start(ck_sb[:, :], ck_d[:, :])
            cvb_sb = consts.tile([128, D], F32)
            nc.sync.dma_start(cvb_sb[:, :], cvb_d[:, :])
            negI_sb = consts.tile([128, 128], F8)
            nc.sync.dma_start(negI_sb[:, :], negI_d[:, :])

            # ---- persistent activations ----
            qT = persist.tile([4 * D, N_PAD], F16)
            kT = persist.tile([4 * D, N_PAD], F16)
            v_sb = persist.tile([128, NJ, 48], F8)  # [j%128, jt, d | ones | pad]
            nc.any.memset(v_sb[:, :, :], 0.0)
            nc.any.memset(v_sb[:, :, D], 1.0)
            p_store = persist.tile([128, NJ, WSZ], F8)

            with (
                # one pool per score-PSUM pair: hazard tracking is
                # tensor-granular, so ring slots must be distinct tensors or
                # the PE serializes behind the previous group's consumer
                tc.tile_pool(name="sp0", bufs=1, space="PSUM") as sp0_pool,
                tc.tile_pool(name="sp1", bufs=1, space="PSUM") as sp1_pool,
                tc.tile_pool(name="av_ps", bufs=2, space="PSUM") as av_pool,
                tc.tile_pool(name="slab", bufs=2) as slab_pool,
                tc.tile_pool(name="out_sb", bufs=3) as out_pool,
            ):
                sp0 = sp0_pool.tile([128, 2, 512], F32)
                sp1 = sp1_pool.tile([128, 2, 512], F32)
                pairs = [sp0, sp1]
                deferred_and = []  # stripe-0 AND-mask groups, issued after the proj adds

                def score_pair(w, gp, pair_a, pair_b, slab_t):
                    """scores + mask + softmax for stripe w, groups gp, gp+1.

                    Batched so consecutive matmuls share PE row-group configs
                    (fewer drain-switch bubbles): PV PV | S S S S | M M M M.
                    """
                    i0 = w * WSZ
                    gs = ((gp, pair_a, PAT[w][gp]),
                          (gp + 1, pair_b, PAT[w][gp + 1]))
                    for g, pair, path in gs:
                        for b in range(2):
                            jt = 2 * g + b
                            nc.tensor.matmul(
                                pair[:, b, :],
                                kT[32 * b:32 * b + 32, jt * 128:(jt + 1) * 128],
                                qT[32 * b:32 * b + 32, i0:i0 + WSZ],
                                start=True, stop=(path == AD),
                                skip_group_check=True)
                    for g, pair, path in gs:
                        if path != AD:
                            for b in range(2):
                                jt = 2 * g + b
                                nc.tensor.matmul(
                                    pair[:, b, :], negI_sb[:, :],
                                    slab_t[:, jt, :],
                                    start=False, stop=True,
                                    skip_group_check=True)
                    for g, pair, path in gs:
                        p_dst = p_store[:, 2 * g:2 * g + 2, :]
                        if path == AD and w == 0:
                            nc.scalar.activation(p_dst, pair[:, :, :],
                                                 AF.Exp, scale=float(SCALE))
                            deferred_and.append(g)
                            continue
                        if path == DP:
                            nc.vector.tensor_scalar(
                                p_dst.bitcast(U8), pair[:, :, :],
                                float(A8), float(B8), op0=ALU.mult, op1=ALU.add)
                        else:
                            nc.scalar.activation(p_dst, pair[:, :, :],
                                                 AF.Exp, scale=float(SCALE))
                            if path == AD:
                                # keep-mask as 0xFF/0x00 bytes; AND at u16
                                # gets the DVE 16-bit 2x packing mode
                                nc.vector.tensor_tensor(
                                    p_dst.bitcast(U16), p_dst.bitcast(U16),
                                    slab_t[:, 2 * g:2 * g + 2, :].bitcast(U16),
                                    op=ALU.bitwise_and)

                def epilogue(av_t, wp):
                    i0 = wp * WSZ
                    avsb = out_pool.tile([34, WSZ], F32, name="avsb")
                    if wp % 2 == 0:
                        nc.scalar.copy(avsb[:, :], av_t[:, :])
                    else:
                        nc.vector.tensor_copy(avsb[:, :], av_t[:, :])
                    nc.sync.dma_start(av_d[:, i0:i0 + WSZ], avsb[:, :])

                # ---- PE warmup: junk matmuls while input DMAs are in flight.
                # ~3.4us of sustained PE-busy flips the HAM clock gate to
                # 2.4 GHz before the real work lands (saves ~5us of cold-rate
                # projections/scores at the start).
                for i in range(44):
                    nc.tensor.matmul(sp0[:, 0, 0:128], negI_sb[:, :],
                                     negI_sb[:, :], start=True, stop=True)

                # ---- phase 0: projections interleaved with stripe-0 scores ----
                slab0 = slab_pool.tile([128, NJ, WSZ], F8, name="slab0")
                nc.sync.dma_start(slab0[:, 0:4, :], nm3_d[:, 0, 0:4, :])
                with (
                    tc.tile_pool(name="proj_in", bufs=8) as proj_in,
                    tc.tile_pool(name="proj_ps", bufs=2, space="PSUM") as proj_ps,
                ):
                    for t in range(NT):
                        n0 = t * 512
                        img_x = proj_in.tile([128, 2, 512], F16, name="img_x")
                        img_c = proj_in.tile([128, 2, 512], F16, name="img_c")
                        for ct in range(2):
                            nc.sync.dma_start(
                                img_x[:, ct, :],
                                xp_d[ct * 128:(ct + 1) * 128, n0:n0 + 512])
                            nc.sync.dma_start(
                                img_c[:, ct, :],
                                cp_d[ct * 128:(ct + 1) * 128, n0:n0 + 512])
                        # prefetch stripe-0 mask chunks for the NEXT t
                        if t < NT - 1:
                            j0 = 4 * t + 4
                            nc.sync.dma_start(slab0[:, j0:j0 + 4, :],
                                              nm3_d[:, 0, j0:j0 + 4, :])
                        qps = proj_ps.tile([128, 512], F32, name="qps")
                        for ct in range(2):
                            nc.tensor.matmul(qps[:, :], AqT_sb[:, ct, :],
                                             img_x[:, ct, :],
                                             start=(ct == 0), stop=(ct == 1))
                        kps = proj_ps.tile([128, 512], F32, name="qps")
                        for ct in range(2):
                            nc.tensor.matmul(kps[:, :], AkT_sb[:, ct, :],
                                             img_c[:, ct, :],
                                             start=(ct == 0), stop=(ct == 1))
                        for jj in range(4):
                            jt = 4 * t + jj
                            vps = av_pool.tile([128, D], F32, name="av")
                            for ct in range(2):
                                nc.tensor.matmul(
                                    vps[:, :],
                                    img_c[:, ct, jj * 128:(jj + 1) * 128],
                                    AvT_sb[:, ct, :],
                                    start=(ct == 0), stop=(ct == 1))
                            nc.vector.tensor_add(v_sb[:, jt, 0:D], vps[:, :],
                                                 cvb_sb[:, :])
                        # stripe-0 scoring lags the projections by 2 tiles and
                        # is issued BEFORE this tile's q/k bias adds: hazard
                        # tracking on qT/kT is tensor-granular, so the scores'
                        # recorded dep is then an add that already completed,
                        # not one still queued behind this tile's DVE work.
                        if t >= 2:
                            score_pair(0, 2 * (t - 2), pairs[0], pairs[1],
                                       slab0)
                        nc.vector.tensor_scalar_add(qT[:, n0:n0 + 512], qps[:, :],
                                                    cq_sb[:, :])
                        nc.vector.tensor_scalar_add(kT[:, n0:n0 + 512], kps[:, :],
                                                    ck_sb[:, :])
                    # drain the two lagged stripe-0 score pairs
                    for tt in (NT - 2, NT - 1):
                        score_pair(0, 2 * tt, pairs[0], pairs[1], slab0)
                    # stripe-0 AND-masks, deferred so the DVE FIFO served all
                    # q/k adds (which gate the lagged scoring) first
                    for g in deferred_and:
                        p_dst = p_store[:, 2 * g:2 * g + 2, :]
                        nc.vector.tensor_tensor(
                            p_dst.bitcast(U16), p_dst.bitcast(U16),
                            slab0[:, 2 * g:2 * g + 2, :].bitcast(U16),
                            op=ALU.bitwise_and)

                # ---- main loop: score stripe w, PV stripe w-1 ----
                with tc.tile_pool(name="sp2", bufs=1, space="PSUM") as sp2_pool:
                    sp2 = sp2_pool.tile([128, 2, 512], F32)
                    pairs3 = [sp0, sp1, sp2]
                    gctr = 0
                    pending = None
                    for w in range(1, NW + 1):
                        if w < NW:
                            slab = slab_pool.tile([128, NJ, WSZ], F8, name="slab")
                            nc.sync.dma_start(slab[:, :, :], nm3_d[:, w, :, :])
                        av = av_pool.tile([34, WSZ], F32, name="av")
                        for gp in range(0, NG, 2):
                            for g in (gp, gp + 1):
                                nc.tensor.matmul(
                                    av[:, :],
                                    v_sb[:, 2 * g:2 * g + 2, 0:34],
                                    p_store[:, 2 * g:2 * g + 2, :],
                                    start=(g == 0), stop=(g == NG - 1),
                                    perf_mode=DR)
                            if w < NW:
                                score_pair(w, gp, pairs3[gctr % 3],
                                           pairs3[(gctr + 1) % 3], slab)
                                gctr += 2
                            if gp == 4 and pending is not None:
                                epilogue(*pending)
                                pending = None
                        pending = (av, w - 1)
                    epilogue(*pending)

    nc.compile()
    return nc


def make_pos(row_embed, col_embed):
    """[EMB, H*W]; first half col embeds, second half row embeds."""
    d2 = row_embed.shape[1]
    pos = np.empty((EMB, H, W), np.float32)
    pos[:d2] = col_embed[:W].T[:, None, :]      # [d2, 1, W] -> broadcast H
    pos[d2:] = row_embed[:H].T[:, :, None]      # [d2, H, 1] -> broadcast W
    return pos.reshape(EMB, H * W)


def make_in_maps(x, context, pad_mask, row_embed, col_embed, w_in, b_in,
                 wq, bq, wk, bk, wv, bv, n_heads=HEADS):
    f8 = np.float64
    x = np.asarray(x, np.float32)
    context = np.asarray(context, np.float32)
    pad_mask = np.asarray(pad_mask)
    row_embed = np.asarray(row_embed, np.float32)
    col_embed = np.asarray(col_embed, np.float32)
    w_in = np.asarray(w_in, f8)
    b_in = np.asarray(b_in, f8)
    wq, bq = np.asarray(wq, f8), np.asarray(bq, f8)
    wk, bk = np.asarray(wk, f8), np.asarray(bk, f8)
    wv, bv = np.asarray(wv, f8), np.asarray(bv, f8)

    import ml_dtypes
    pos = make_pos(row_embed, col_embed)
    xp = np.zeros((EMB, N_PAD), np.float16)
    xp[:, :N_TOK] = (x.reshape(EMB, N_TOK) + pos).astype(np.float16)
    cp = np.zeros((EMB, N_PAD), np.float16)
    cp[:, :N_TOK] = (context.reshape(EMB, N_TOK) + pos).astype(np.float16)
    # additive mask [j, i]: 1.0 where attention is masked (or j padded)
    am = np.zeros((N_PAD, N_PAD), np.float32)
    am[:N_TOK, :N_TOK] = pad_mask[0].T
    am[N_TOK:, :] = 1.0
    nm4 = np.ascontiguousarray(
        am.reshape(NJ, 128, NW, WSZ).transpose(1, 2, 0, 3))  # [128, NW, NJ, WSZ]
    nm3 = nm4.astype(ml_dtypes.float8_e4m3)
    # AD groups get a byte-level keep-mask (0xFF keep / 0x00 drop) which the
    # DVE ANDs onto the fp8 probabilities
    u8v = nm3.view(np.uint8)
    for w in range(NW):
        for g in range(NG):
            if PAT[w][g] == AD:
                u8v[:, w, 2 * g:2 * g + 2, :] = np.where(
                    nm4[:, w, 2 * g:2 * g + 2, :] > 0.5, 0, 0xFF)
    negI = (-NEG * np.eye(128, dtype=np.float32)).astype(ml_dtypes.float8_e4m3)

    shared = {"xp": xp, "cp": cp, "nm3": nm3, "negI": negI}
    in_maps = []
    for h in range(n_heads):
        sl = slice(h * DEPTH, (h + 1) * DEPTH)
        Aq = wq[sl] @ w_in          # [D, IN_CH]
        cq = wq[sl] @ b_in + bq[sl]
        Ak = wk[sl] @ w_in
        ck = wk[sl] @ b_in + bk[sl]
        Av = wv[sl] @ w_in
        cv = wv[sl] @ b_in + bv[sl]
        f32c = lambda a: np.ascontiguousarray(a.astype(np.float32))
        in_maps.append(dict(
            shared,
            AqT=np.ascontiguousarray(np.tile(Aq.T, (1, 4)).astype(np.float16)),
            cq=f32c(np.tile(cq.reshape(DEPTH, 1), (4, 1))),
            AkT=np.ascontiguousarray(np.tile(Ak.T, (1, 4)).astype(np.float16)),
            ck=f32c(np.tile(ck.reshape(DEPTH, 1), (4, 1))),
            AvT=np.ascontiguousarray(Av.T.astype(np.float16)),
            cvb=f32c(np.broadcast_to(cv, (128, DEPTH))),
        ))
    return in_maps


_CACHE = {}


def kernel(x, context, pad_mask, row_embed, col_embed, w_in, b_in,
           wq, bq, wk, bk, wv, bv, w_out, b_out):
    if "nc" not in _CACHE:
        _CACHE["nc"] = build_nc()
    nc = _CACHE["nc"]
    in_maps = make_in_maps(x, context, pad_mask, row_embed, col_embed,
                           w_in, b_in, wq, bq, wk, bk, wv, bv)
    res = run_bass_kernel_spmd(nc, in_maps, core_ids=list(range(HEADS)))
    w_out = np.asarray(w_out, np.float32)
    y = np.zeros((EMB, N_TOK), np.float64)
    for h in range(HEADS):
        av = res.results[h]["av"]                       # [34, N_PAD] f32
        num = av[0:DEPTH, :N_TOK].astype(np.float64)
        den = av[DEPTH, :N_TOK].astype(np.float64)
        out_h = (num / den).astype(np.float32)
        y += (w_out[:, h * DEPTH:(h + 1) * DEPTH] @ out_h).astype(np.float64)
    y = (y + np.asarray(b_out, np.float64)[:, None]).astype(np.float32)
    return y.reshape(EMB, H, W)
